# revision 9
# baseline (speedup 1.0000x reference)
"""Multi-head attention (B=2, S=2048, D=1024, H=16) on 8 trn2 NeuronCores.

Sharding: 2-way batch x 4-way heads (core c -> batch c//4, heads (c%4)*4..+4).
Each core:
  - projects its batch's full query/key/value (passed transposed, [D, S])
    against its 256-column slice of Wq/Wk/Wv -> Q^T, K^T [256, S], V [S, 256]
  - per head: S^T = K_h Q_h^T (scores transposed), E^T = exp(S^T/8),
    ctx^T/sumexp via one matmul against V augmented with a ones column,
    normalized ctx via a gpsimd partition-broadcast of 1/sumexp
  - out_partial = ctx_norm @ Wo[rows of this core's heads]  [S, 1024]
Outputs per core: E^T per head [4, S, S], recip = 1/sumexp [4, S], out_partial.
Host: att[b,h] = (E^T * recip).T (normalize + un-transpose),
      out[b] = sum of 4 cores' partials + bo + bv @ Wo (exact bias folding).
bq/bk are added on-device during the Q^T/K^T PSUM->SBUF copy (per-partition).
"""

import numpy as np

import concourse.bacc as bacc
import concourse.tile as tile
from concourse import mybir
from concourse import bass_utils

N_CORES = 8
B, S, D = 2, 2048, 1024
H, DH = 16, 64
HPC = 4            # heads per core
CW = HPC * DH      # column slice width per core = 256
F32 = mybir.dt.float32
F32R = mybir.dt.float32r

# matmul input dtype per stage (float32r = 4x faster fp32 path on PE)
DT_PROJ = F32R
DT_SCORE = F32R
DT_AV = F32R
DT_WO = F32R

PCHUNK = 512       # phase-1 column chunk (rows of X) per step
NPC = S // PCHUNK  # 4 chunks
QC = 512           # phase-2 q chunk
NQC = S // QC      # 4
KT = S // 128      # 16 k tiles


def _mm(nc, out, lhsT, rhs, dt, **kw):
    nc.tensor.matmul(out, lhsT.bitcast(dt), rhs.bitcast(dt), **kw)


def build_nc():
    nc = bacc.Bacc("TRN2", target_bir_lowering=False, debug=False,
                   num_devices=N_CORES)

    xqT = nc.dram_tensor("xqT", [D, S], F32R, kind="ExternalInput").ap()
    xkT = nc.dram_tensor("xkT", [D, S], F32R, kind="ExternalInput").ap()
    xvT = nc.dram_tensor("xvT", [D, S], F32R, kind="ExternalInput").ap()
    wq = nc.dram_tensor("wq", [D, CW], F32R, kind="ExternalInput").ap()
    wk = nc.dram_tensor("wk", [D, CW], F32R, kind="ExternalInput").ap()
    wv = nc.dram_tensor("wv", [D, CW], F32R, kind="ExternalInput").ap()
    bq = nc.dram_tensor("bq", [CW], F32, kind="ExternalInput").ap()
    bk = nc.dram_tensor("bk", [CW], F32, kind="ExternalInput").ap()
    wo = nc.dram_tensor("wo", [CW, D], F32R, kind="ExternalInput").ap()

    et_out = nc.dram_tensor("et", [HPC, S, S], F32R, kind="ExternalOutput").ap()
    recip_out = nc.dram_tensor("recip", [HPC, S], F32, kind="ExternalOutput").ap()
    outp = nc.dram_tensor("outp", [S, D], F32, kind="ExternalOutput").ap()

    with tile.TileContext(nc) as tc:
        _emit(nc, tc, xqT, xkT, xvT, wq, wk, wv, bq, bk, wo,
              et_out, recip_out, outp)
    nc.compile()
    return nc


def _emit(nc, tc, xqT, xkT, xvT, wq, wk, wv, bq, bk, wo,
          et_out, recip_out, outp):
    with (
        tc.tile_pool(name="persist", bufs=1) as persist,
        tc.tile_pool(name="xchunks", bufs=3) as xchunks,
        tc.tile_pool(name="work", bufs=3) as work,
        tc.tile_pool(name="ps1", bufs=3, space="PSUM") as ps1,
        tc.tile_pool(name="pst", bufs=3, space="PSUM") as pstp,
        tc.tile_pool(name="psctx", bufs=2, space="PSUM") as psctx,
    ):
        # ---- persistent SBUF ----
        wq_sb = persist.tile([128, 8, CW], F32R, tag="wq")
        wk_sb = persist.tile([128, 8, CW], F32R, tag="wk")
        wv_sb = persist.tile([128, 8, CW], F32R, tag="wv")
        wo_sb = persist.tile([128, 2, D], F32R, tag="wo")
        bq_sb = persist.tile([128, 2], F32, tag="bq")
        bk_sb = persist.tile([128, 2], F32, tag="bk")
        qt2 = [persist.tile([128, S], F32R, tag=f"qt{m}", name=f"qt{m}") for m in range(2)]
        kt2 = [persist.tile([128, S], F32R, tag=f"kt{m}", name=f"kt{m}") for m in range(2)]
        vb = [persist.tile([128, KT, DH + 1], F32R, tag=f"vb{h}", name=f"vb{h}")
              for h in range(HPC)]
        ctxT = [persist.tile([128, S], F32R, tag=f"ctxT{m}", name=f"ctxT{m}") for m in range(2)]

        nc.scalar.dma_start(out=wq_sb, in_=wq.rearrange("(k p) n -> p k n", p=128))
        nc.scalar.dma_start(out=wk_sb, in_=wk.rearrange("(k p) n -> p k n", p=128))
        nc.scalar.dma_start(out=wv_sb, in_=wv.rearrange("(k p) n -> p k n", p=128))
        nc.scalar.dma_start(out=wo_sb, in_=wo.rearrange("(m p) n -> p m n", p=128))
        nc.scalar.dma_start(out=bq_sb, in_=bq.rearrange("(m p) -> p m", p=128))
        nc.scalar.dma_start(out=bk_sb, in_=bk.rearrange("(m p) -> p m", p=128))
        ones_sc = persist.tile([128, KT], F32, tag="ones")
        nc.vector.memset(ones_sc, 1.0)
        for h in range(HPC):
            nc.vector.tensor_copy(out=vb[h][:, :, DH], in_=ones_sc)

        # ---- phase 1: projections (pipelined k/v/q chunk order) ----
        with nc.named_scope("proj"):
            for c in range(NPC):
                sl = slice(c * PCHUNK, (c + 1) * PCHUNK)
                xk_c = xchunks.tile([128, 8, PCHUNK], F32R, tag="x", name=f"xk{c}")
                nc.scalar.dma_start(
                    out=xk_c, in_=xkT[:, sl].rearrange("(k p) n -> p k n", p=128))
                for m in range(2):
                    msl = slice(m * 128, (m + 1) * 128)
                    pk = ps1.tile([128, PCHUNK], F32, tag="p1", name=f"pk{c}{m}")
                    for k in range(8):
                        _mm(nc, pk, wk_sb[:, k, msl], xk_c[:, k, :], DT_PROJ,
                            start=(k == 0), stop=(k == 7))
                    nc.vector.tensor_scalar_add(
                        out=kt2[m][:, sl], in0=pk, scalar1=bk_sb[:, m:m + 1])
                xv_c = xchunks.tile([128, 8, PCHUNK], F32R, tag="x", name=f"xv{c}")
                nc.scalar.dma_start(
                    out=xv_c, in_=xvT[:, sl].rearrange("(k p) n -> p k n", p=128))
                for r in range(PCHUNK // 128):
                    rt = c * (PCHUNK // 128) + r
                    rsl = slice(r * 128, (r + 1) * 128)
                    pv = ps1.tile([128, CW], F32, tag="p1", name=f"pv{c}{r}")
                    for k in range(8):
                        _mm(nc, pv, xv_c[:, k, rsl], wv_sb[:, k, :], DT_PROJ,
                            start=(k == 0), stop=(k == 7))
                    for h in range(HPC):
                        nc.vector.tensor_copy(
                            out=vb[h][:, rt, 0:DH],
                            in_=pv[:, h * DH:(h + 1) * DH])
                xq_c = xchunks.tile([128, 8, PCHUNK], F32R, tag="x", name=f"xq{c}")
                nc.scalar.dma_start(
                    out=xq_c, in_=xqT[:, sl].rearrange("(k p) n -> p k n", p=128))
                for m in range(2):
                    msl = slice(m * 128, (m + 1) * 128)
                    pq = ps1.tile([128, PCHUNK], F32, tag="p1", name=f"pq{c}{m}")
                    for k in range(8):
                        _mm(nc, pq, wq_sb[:, k, msl], xq_c[:, k, :], DT_PROJ,
                            start=(k == 0), stop=(k == 7))
                    nc.vector.tensor_scalar_add(
                        out=qt2[m][:, sl], in0=pq, scalar1=bq_sb[:, m:m + 1])

        # ---- phase 2+3: attention (qc outer) + output projection ----
        with nc.named_scope("attn"):
            for qc in range(NQC):
                qsl = slice(qc * QC, (qc + 1) * QC)
                for h in range(HPC):
                    m = h // 2
                    ho = (h % 2) * DH
                    hsl = slice(ho, ho + DH)
                    pctx = psctx.tile([DH + 1, QC], F32, tag="ctx",
                                      name=f"ctx{qc}{h}")
                    for kt in range(KT):
                        ksl = slice(kt * 128, (kt + 1) * 128)
                        pst = pstp.tile([128, QC], F32, tag="pst",
                                        name=f"pst{qc}{h}{kt}")
                        _mm(nc, pst, kt2[m][hsl, ksl], qt2[m][hsl, qsl],
                            DT_SCORE, start=True, stop=True)
                        et_t = work.tile([128, QC], F32R, tag="et",
                                         name=f"et{qc}{h}{kt}")
                        nc.scalar.activation(
                            out=et_t, in_=pst,
                            func=mybir.ActivationFunctionType.Exp, scale=0.125)
                        nc.sync.dma_start(out=et_out[h, ksl, qsl], in_=et_t)
                        _mm(nc, pctx, vb[h][:, kt, :], et_t, DT_AV,
                            start=(kt == 0), stop=(kt == KT - 1))
                    rrow = work.tile([1, QC], F32, tag="rrow", name=f"rr{qc}{h}")
                    nc.vector.reciprocal(out=rrow, in_=pctx[DH:DH + 1, :])
                    nc.gpsimd.dma_start(out=recip_out[h, qsl], in_=rrow)
                    rs = work.tile([DH, QC], F32, tag="rs", name=f"rs{qc}{h}")
                    nc.gpsimd.partition_broadcast(rs, rrow)
                    nc.vector.tensor_mul(
                        out=ctxT[m][hsl, qsl], in0=pctx[0:DH, :], in1=rs)
                with nc.named_scope("wo"):
                    for q2 in range(QC // 128):
                        qt = qc * (QC // 128) + q2
                        qtsl = slice(qt * 128, (qt + 1) * 128)
                        o_t = work.tile([128, D], F32, tag="ot", name=f"ot{qt}")
                        for n2 in range(2):
                            nsl = slice(n2 * 512, (n2 + 1) * 512)
                            po = pstp.tile([128, 512], F32, tag="pst",
                                           name=f"po{qt}{n2}")
                            for m2 in range(2):
                                _mm(nc, po, ctxT[m2][:, qtsl],
                                    wo_sb[:, m2, nsl], DT_WO,
                                    start=(m2 == 0), stop=(m2 == 1))
                            nc.vector.tensor_copy(out=o_t[:, nsl], in_=po)
                        nc.gpsimd.dma_start(out=outp[qtsl, :], in_=o_t)


_NC_CACHE = None
LAST_RESULTS = None


def _get_nc():
    global _NC_CACHE
    if _NC_CACHE is None:
        _NC_CACHE = build_nc()
    return _NC_CACHE


def kernel(query, key, value, Wq, bq, Wk, bk, Wv, bv, Wo, bo):
    query = np.asarray(query, dtype=np.float32)
    key = np.asarray(key, dtype=np.float32)
    value = np.asarray(value, dtype=np.float32)
    Wq, Wk, Wv, Wo = (np.asarray(a, dtype=np.float32) for a in (Wq, Wk, Wv, Wo))
    bq, bk, bv, bo = (np.asarray(a, dtype=np.float32) for a in (bq, bk, bv, bo))

    nc = _get_nc()

    xT = {}
    for b in range(B):
        xT[b] = (np.ascontiguousarray(query[b].T),
                 np.ascontiguousarray(key[b].T),
                 np.ascontiguousarray(value[b].T))

    in_maps = []
    for c in range(N_CORES):
        b = c // 4
        cb = (c % 4) * CW
        csl = slice(cb, cb + CW)
        in_maps.append({
            "xqT": xT[b][0], "xkT": xT[b][1], "xvT": xT[b][2],
            "wq": np.ascontiguousarray(Wq[:, csl]),
            "wk": np.ascontiguousarray(Wk[:, csl]),
            "wv": np.ascontiguousarray(Wv[:, csl]),
            "bq": np.ascontiguousarray(bq[csl]),
            "bk": np.ascontiguousarray(bk[csl]),
            "wo": np.ascontiguousarray(Wo[csl, :]),
        })

    global LAST_RESULTS
    res = bass_utils.run_bass_kernel_spmd(
        nc, in_maps, core_ids=list(range(N_CORES)))
    LAST_RESULTS = res

    att = np.empty((B, H, S, S), dtype=np.float32)
    out = np.zeros((B, S, D), dtype=np.float32)
    for c in range(N_CORES):
        b = c // 4
        r = res.results[c]
        et = r["et"]          # [HPC, S(k), S(q)]
        recip = r["recip"]    # [HPC, S(q)]
        for hl in range(HPC):
            h = (c % 4) * HPC + hl
            att[b, h] = (et[hl] * recip[hl]).T
        out[b] += r["outp"]
    out += bo + bv @ Wo
    return out, att


# revision 10
# speedup vs baseline: 1.0036x; 1.0036x over previous
"""Multi-head attention (B=2, S=2048, D=1024, H=16) on 8 trn2 NeuronCores.

Sharding: 2-way batch x 4-way heads (core c -> batch c//4, heads (c%4)*4..+4).
Each core:
  - projects its batch's full query/key/value (passed transposed, [D, S])
    against its 256-column slice of Wq/Wk/Wv -> Q^T, K^T [256, S], V [S, 256]
  - per head: S^T = K_h Q_h^T (scores transposed), E^T = exp(S^T/8),
    ctx^T/sumexp via one matmul against V augmented with a ones column,
    normalized ctx via a gpsimd partition-broadcast of 1/sumexp
  - out_partial = ctx_norm @ Wo[rows of this core's heads]  [S, 1024]
Outputs per core: E^T per head [4, S, S], recip = 1/sumexp [4, S], out_partial.
Host: att[b,h] = (E^T * recip).T (normalize + un-transpose),
      out[b] = sum of 4 cores' partials + bo + bv @ Wo (exact bias folding).
bq/bk are added on-device during the Q^T/K^T PSUM->SBUF copy (per-partition).
"""

import numpy as np

import concourse.bacc as bacc
import concourse.tile as tile
from concourse import mybir
from concourse import bass_utils

N_CORES = 8
B, S, D = 2, 2048, 1024
H, DH = 16, 64
HPC = 4            # heads per core
CW = HPC * DH      # column slice width per core = 256
F32 = mybir.dt.float32
F32R = mybir.dt.float32r

# matmul input dtype per stage (float32r = 4x faster fp32 path on PE)
DT_PROJ = F32R
DT_SCORE = F32R
DT_AV = F32R
DT_WO = F32R

PCHUNK = 512       # phase-1 column chunk (rows of X) per step
NPC = S // PCHUNK  # 4 chunks
QC = 512           # phase-2 q chunk
NQC = S // QC      # 4
KT = S // 128      # 16 k tiles


def _mm(nc, out, lhsT, rhs, dt, **kw):
    nc.tensor.matmul(out, lhsT.bitcast(dt), rhs.bitcast(dt), **kw)


def build_nc():
    nc = bacc.Bacc("TRN2", target_bir_lowering=False, debug=False,
                   num_devices=N_CORES)

    xqT = nc.dram_tensor("xqT", [D, S], F32R, kind="ExternalInput").ap()
    xkT = nc.dram_tensor("xkT", [D, S], F32R, kind="ExternalInput").ap()
    xvT = nc.dram_tensor("xvT", [D, S], F32R, kind="ExternalInput").ap()
    wq = nc.dram_tensor("wq", [D, CW], F32R, kind="ExternalInput").ap()
    wk = nc.dram_tensor("wk", [D, CW], F32R, kind="ExternalInput").ap()
    wv = nc.dram_tensor("wv", [D, CW], F32R, kind="ExternalInput").ap()
    bq = nc.dram_tensor("bq", [CW], F32, kind="ExternalInput").ap()
    bk = nc.dram_tensor("bk", [CW], F32, kind="ExternalInput").ap()
    wo = nc.dram_tensor("wo", [CW, D], F32R, kind="ExternalInput").ap()

    et_out = nc.dram_tensor("et", [HPC, S, S], F32R, kind="ExternalOutput").ap()
    recip_out = nc.dram_tensor("recip", [HPC, S], F32, kind="ExternalOutput").ap()
    outp = nc.dram_tensor("outp", [S, D], F32, kind="ExternalOutput").ap()

    with tile.TileContext(nc) as tc:
        _emit(nc, tc, xqT, xkT, xvT, wq, wk, wv, bq, bk, wo,
              et_out, recip_out, outp)
    nc.compile()
    return nc


def _emit(nc, tc, xqT, xkT, xvT, wq, wk, wv, bq, bk, wo,
          et_out, recip_out, outp):
    with (
        tc.tile_pool(name="persist", bufs=1) as persist,
        tc.tile_pool(name="xchunks", bufs=3) as xchunks,
        tc.tile_pool(name="work", bufs=3) as work,
        tc.tile_pool(name="pst", bufs=4, space="PSUM") as pstp,
        tc.tile_pool(name="psctx", bufs=4, space="PSUM") as psctx,
    ):
        # ---- persistent SBUF ----
        wq_sb = persist.tile([128, 8, CW], F32R, tag="wq")
        wk_sb = persist.tile([128, 8, CW], F32R, tag="wk")
        wv_sb = persist.tile([128, 8, CW], F32R, tag="wv")
        wo_sb = persist.tile([128, 2, D], F32R, tag="wo")
        bq_sb = persist.tile([128, 2], F32, tag="bq")
        bk_sb = persist.tile([128, 2], F32, tag="bk")
        qt2 = [persist.tile([128, S], F32R, tag=f"qt{m}", name=f"qt{m}") for m in range(2)]
        kt2 = [persist.tile([128, S], F32R, tag=f"kt{m}", name=f"kt{m}") for m in range(2)]
        vb = [persist.tile([128, KT, DH + 1], F32R, tag=f"vb{h}", name=f"vb{h}")
              for h in range(HPC)]
        ctxT = [persist.tile([128, S], F32R, tag=f"ctxT{m}", name=f"ctxT{m}") for m in range(2)]

        nc.scalar.dma_start(out=wq_sb, in_=wq.rearrange("(k p) n -> p k n", p=128))
        nc.scalar.dma_start(out=wk_sb, in_=wk.rearrange("(k p) n -> p k n", p=128))
        nc.scalar.dma_start(out=wv_sb, in_=wv.rearrange("(k p) n -> p k n", p=128))
        nc.scalar.dma_start(out=wo_sb, in_=wo.rearrange("(m p) n -> p m n", p=128))
        nc.scalar.dma_start(out=bq_sb, in_=bq.rearrange("(m p) -> p m", p=128))
        nc.scalar.dma_start(out=bk_sb, in_=bk.rearrange("(m p) -> p m", p=128))
        ones_sc = persist.tile([128, KT], F32, tag="ones")
        nc.vector.memset(ones_sc, 1.0)
        for h in range(HPC):
            nc.vector.tensor_copy(out=vb[h][:, :, DH], in_=ones_sc)

        # ---- phase 1: projections (pipelined k/v/q chunk order) ----
        with nc.named_scope("proj"):
            for c in range(NPC):
                sl = slice(c * PCHUNK, (c + 1) * PCHUNK)
                xk_c = xchunks.tile([128, 8, PCHUNK], F32R, tag="x", name=f"xk{c}")
                nc.scalar.dma_start(
                    out=xk_c, in_=xkT[:, sl].rearrange("(k p) n -> p k n", p=128))
                for m in range(2):
                    msl = slice(m * 128, (m + 1) * 128)
                    pk = pstp.tile([128, PCHUNK], F32, tag="pst", name=f"pk{c}{m}")
                    for k in range(8):
                        _mm(nc, pk, wk_sb[:, k, msl], xk_c[:, k, :], DT_PROJ,
                            start=(k == 0), stop=(k == 7))
                    nc.vector.tensor_scalar_add(
                        out=kt2[m][:, sl], in0=pk, scalar1=bk_sb[:, m:m + 1])
                xv_c = xchunks.tile([128, 8, PCHUNK], F32R, tag="x", name=f"xv{c}")
                nc.scalar.dma_start(
                    out=xv_c, in_=xvT[:, sl].rearrange("(k p) n -> p k n", p=128))
                for r in range(PCHUNK // 128):
                    rt = c * (PCHUNK // 128) + r
                    rsl = slice(r * 128, (r + 1) * 128)
                    pv = pstp.tile([128, CW], F32, tag="pst", name=f"pv{c}{r}")
                    for k in range(8):
                        _mm(nc, pv, xv_c[:, k, rsl], wv_sb[:, k, :], DT_PROJ,
                            start=(k == 0), stop=(k == 7))
                    for h in range(HPC):
                        nc.vector.tensor_copy(
                            out=vb[h][:, rt, 0:DH],
                            in_=pv[:, h * DH:(h + 1) * DH])
                xq_c = xchunks.tile([128, 8, PCHUNK], F32R, tag="x", name=f"xq{c}")
                nc.scalar.dma_start(
                    out=xq_c, in_=xqT[:, sl].rearrange("(k p) n -> p k n", p=128))
                for m in range(2):
                    msl = slice(m * 128, (m + 1) * 128)
                    pq = pstp.tile([128, PCHUNK], F32, tag="pst", name=f"pq{c}{m}")
                    for k in range(8):
                        _mm(nc, pq, wq_sb[:, k, msl], xq_c[:, k, :], DT_PROJ,
                            start=(k == 0), stop=(k == 7))
                    nc.vector.tensor_scalar_add(
                        out=qt2[m][:, sl], in0=pq, scalar1=bq_sb[:, m:m + 1])

        # ---- phase 2+3: attention (qc outer, head-pair ST row-packing) ----
        with nc.named_scope("attn"):
            for qc in range(NQC):
                qsl = slice(qc * QC, (qc + 1) * QC)
                for m in range(2):
                    pctx = [psctx.tile([DH + 1, QC], F32, tag="ctx",
                                       name=f"ctx{qc}{m}{j}") for j in range(2)]
                    for kt in range(KT):
                        ksl = slice(kt * 128, (kt + 1) * 128)
                        pst = [pstp.tile([128, QC], F32, tag="pst",
                                         name=f"pst{qc}{m}{kt}{j}")
                               for j in range(2)]
                        for j in range(2):
                            hsl = slice(j * DH, (j + 1) * DH)
                            _mm(nc, pst[j], kt2[m][hsl, ksl], qt2[m][hsl, qsl],
                                DT_SCORE, start=True, stop=True,
                                tile_position=(j * DH, 0))
                        for j in range(2):
                            h = 2 * m + j
                            et_t = work.tile([128, QC], F32R, tag="et",
                                             name=f"et{qc}{m}{kt}{j}")
                            nc.scalar.activation(
                                out=et_t, in_=pst[j],
                                func=mybir.ActivationFunctionType.Exp,
                                scale=0.125)
                            nc.sync.dma_start(out=et_out[h, ksl, qsl], in_=et_t)
                            _mm(nc, pctx[j], vb[h][:, kt, :], et_t, DT_AV,
                                start=(kt == 0), stop=(kt == KT - 1))
                    for j in range(2):
                        h = 2 * m + j
                        hsl = slice(j * DH, (j + 1) * DH)
                        rrow = work.tile([1, QC], F32, tag="rrow",
                                         name=f"rr{qc}{m}{j}")
                        nc.vector.reciprocal(out=rrow, in_=pctx[j][DH:DH + 1, :])
                        nc.gpsimd.dma_start(out=recip_out[h, qsl], in_=rrow)
                        rs = work.tile([DH, QC], F32, tag="rs",
                                       name=f"rs{qc}{m}{j}")
                        nc.gpsimd.partition_broadcast(rs, rrow)
                        nc.vector.tensor_mul(
                            out=ctxT[m][hsl, qsl], in0=pctx[j][0:DH, :], in1=rs)
                with nc.named_scope("wo"):
                    for q2 in range(QC // 128):
                        qt = qc * (QC // 128) + q2
                        qtsl = slice(qt * 128, (qt + 1) * 128)
                        o_t = work.tile([128, D], F32, tag="ot", name=f"ot{qt}")
                        for n2 in range(2):
                            nsl = slice(n2 * 512, (n2 + 1) * 512)
                            po = pstp.tile([128, 512], F32, tag="pst",
                                           name=f"po{qt}{n2}")
                            for m2 in range(2):
                                _mm(nc, po, ctxT[m2][:, qtsl],
                                    wo_sb[:, m2, nsl], DT_WO,
                                    start=(m2 == 0), stop=(m2 == 1))
                            nc.vector.tensor_copy(out=o_t[:, nsl], in_=po)
                        nc.gpsimd.dma_start(out=outp[qtsl, :], in_=o_t)


_NC_CACHE = None
LAST_RESULTS = None


def _get_nc():
    global _NC_CACHE
    if _NC_CACHE is None:
        _NC_CACHE = build_nc()
    return _NC_CACHE


def kernel(query, key, value, Wq, bq, Wk, bk, Wv, bv, Wo, bo):
    query = np.asarray(query, dtype=np.float32)
    key = np.asarray(key, dtype=np.float32)
    value = np.asarray(value, dtype=np.float32)
    Wq, Wk, Wv, Wo = (np.asarray(a, dtype=np.float32) for a in (Wq, Wk, Wv, Wo))
    bq, bk, bv, bo = (np.asarray(a, dtype=np.float32) for a in (bq, bk, bv, bo))

    nc = _get_nc()

    xT = {}
    for b in range(B):
        xT[b] = (np.ascontiguousarray(query[b].T),
                 np.ascontiguousarray(key[b].T),
                 np.ascontiguousarray(value[b].T))

    in_maps = []
    for c in range(N_CORES):
        b = c // 4
        cb = (c % 4) * CW
        csl = slice(cb, cb + CW)
        in_maps.append({
            "xqT": xT[b][0], "xkT": xT[b][1], "xvT": xT[b][2],
            "wq": np.ascontiguousarray(Wq[:, csl]),
            "wk": np.ascontiguousarray(Wk[:, csl]),
            "wv": np.ascontiguousarray(Wv[:, csl]),
            "bq": np.ascontiguousarray(bq[csl]),
            "bk": np.ascontiguousarray(bk[csl]),
            "wo": np.ascontiguousarray(Wo[csl, :]),
        })

    global LAST_RESULTS
    res = bass_utils.run_bass_kernel_spmd(
        nc, in_maps, core_ids=list(range(N_CORES)))
    LAST_RESULTS = res

    att = np.empty((B, H, S, S), dtype=np.float32)
    out = np.zeros((B, S, D), dtype=np.float32)
    for c in range(N_CORES):
        b = c // 4
        r = res.results[c]
        et = r["et"]          # [HPC, S(k), S(q)]
        recip = r["recip"]    # [HPC, S(q)]
        for hl in range(HPC):
            h = (c % 4) * HPC + hl
            att[b, h] = (et[hl] * recip[hl]).T
        out[b] += r["outp"]
    out += bo + bv @ Wo
    return out, att


# revision 11
# speedup vs baseline: 1.2481x; 1.2435x over previous
"""Multi-head attention (B=2, S=2048, D=1024, H=16) on 8 trn2 NeuronCores.

Sharding: 2-way batch x 4-way heads (core c -> batch c//4, heads (c%4)*4..+4).
Each core:
  - projects its batch's full query/key/value (passed transposed, [D, S])
    against its 256-column slice of Wq/Wk/Wv -> Q^T, K^T [256, S], V [S, 256]
  - per head: S^T = K_h Q_h^T (scores transposed), E^T = exp(S^T/8),
    ctx^T/sumexp via one matmul against V augmented with a ones column,
    normalized ctx via a gpsimd partition-broadcast of 1/sumexp
  - out_partial = ctx_norm @ Wo[rows of this core's heads]  [S, 1024]
Outputs per core: E^T per head [4, S, S], recip = 1/sumexp [4, S], out_partial.
Host: att[b,h] = (E^T * recip).T (normalize + un-transpose),
      out[b] = sum of 4 cores' partials + bo + bv @ Wo (exact bias folding).
bq/bk are added on-device during the Q^T/K^T PSUM->SBUF copy (per-partition).
"""

import numpy as np

import concourse.bacc as bacc
import concourse.tile as tile
from concourse import mybir
from concourse import bass_utils

N_CORES = 8
B, S, D = 2, 2048, 1024
H, DH = 16, 64
HPC = 4            # heads per core
CW = HPC * DH      # column slice width per core = 256
F32 = mybir.dt.float32
F32R = mybir.dt.float32r

# matmul input dtype per stage (float32r = 4x faster fp32 path on PE)
DT_PROJ = F32R
DT_SCORE = F32R
DT_AV = F32R
DT_WO = F32R

PCHUNK = 512       # phase-1 column chunk (rows of X) per step
NPC = S // PCHUNK  # 4 chunks
QC = 512           # phase-2 q chunk
NQC = S // QC      # 4
KT = S // 128      # 16 k tiles


def _mm(nc, out, lhsT, rhs, dt, **kw):
    nc.tensor.matmul(out, lhsT.bitcast(dt), rhs.bitcast(dt), **kw)


def build_nc():
    nc = bacc.Bacc("TRN2", target_bir_lowering=False, debug=False,
                   num_devices=N_CORES)

    xqT = nc.dram_tensor("xqT", [D, S], F32R, kind="ExternalInput").ap()
    xkT = nc.dram_tensor("xkT", [D, S], F32R, kind="ExternalInput").ap()
    xvT = nc.dram_tensor("xvT", [D, S], F32R, kind="ExternalInput").ap()
    wq = nc.dram_tensor("wq", [D, CW], F32R, kind="ExternalInput").ap()
    wk = nc.dram_tensor("wk", [D, CW], F32R, kind="ExternalInput").ap()
    wv = nc.dram_tensor("wv", [D, CW], F32R, kind="ExternalInput").ap()
    bq = nc.dram_tensor("bq", [CW], F32, kind="ExternalInput").ap()
    bk = nc.dram_tensor("bk", [CW], F32, kind="ExternalInput").ap()
    wo = nc.dram_tensor("wo", [CW, D], F32R, kind="ExternalInput").ap()

    et_out = nc.dram_tensor("et", [HPC, S, S], F32R, kind="ExternalOutput").ap()
    recip_out = nc.dram_tensor("recip", [HPC, S], F32, kind="ExternalOutput").ap()
    outp = nc.dram_tensor("outp", [S, D], F32, kind="ExternalOutput").ap()

    with tile.TileContext(nc) as tc:
        _emit(nc, tc, xqT, xkT, xvT, wq, wk, wv, bq, bk, wo,
              et_out, recip_out, outp)
    nc.compile()
    return nc


def _emit(nc, tc, xqT, xkT, xvT, wq, wk, wv, bq, bk, wo,
          et_out, recip_out, outp):
    with (
        tc.tile_pool(name="persist", bufs=1) as persist,
        tc.tile_pool(name="xchunks", bufs=2) as xchunks,
        tc.tile_pool(name="work", bufs=3) as work,
        tc.tile_pool(name="pst", bufs=5, space="PSUM") as pstp,
        tc.tile_pool(name="psctx", bufs=3, space="PSUM") as psctx,
    ):
        # ---- persistent SBUF ----
        wq_sb = persist.tile([128, 8, CW], F32R, tag="wq")
        wk_sb = persist.tile([128, 8, CW], F32R, tag="wk")
        wv_sb = persist.tile([128, 8, CW], F32R, tag="wv")
        wo_sb = persist.tile([128, 2, D], F32R, tag="wo")
        bq_sb = persist.tile([128, 2], F32, tag="bq")
        bk_sb = persist.tile([128, 2], F32, tag="bk")
        qt2 = [persist.tile([128, S], F32R, tag=f"qt{m}", name=f"qt{m}") for m in range(2)]
        kt2 = [persist.tile([128, S], F32R, tag=f"kt{m}", name=f"kt{m}") for m in range(2)]
        vb = [persist.tile([128, KT, DH + 1], F32R, tag=f"vb{h}", name=f"vb{h}")
              for h in range(HPC)]
        ctxT = [persist.tile([128, S], F32R, tag=f"ctxT{m}", name=f"ctxT{m}") for m in range(2)]

        nc.scalar.dma_start(out=wq_sb, in_=wq.rearrange("(k p) n -> p k n", p=128))
        nc.scalar.dma_start(out=wk_sb, in_=wk.rearrange("(k p) n -> p k n", p=128))
        nc.scalar.dma_start(out=wv_sb, in_=wv.rearrange("(k p) n -> p k n", p=128))
        nc.scalar.dma_start(out=wo_sb, in_=wo.rearrange("(m p) n -> p m n", p=128))
        nc.scalar.dma_start(out=bq_sb, in_=bq.rearrange("(m p) -> p m", p=128))
        nc.scalar.dma_start(out=bk_sb, in_=bk.rearrange("(m p) -> p m", p=128))
        ones_sc = persist.tile([128, KT], F32, tag="ones")
        nc.vector.memset(ones_sc, 1.0)
        for h in range(HPC):
            nc.vector.tensor_copy(out=vb[h][:, :, DH], in_=ones_sc)

        # ---- phase 1: projections (pipelined k/v/q chunk order) ----
        with nc.named_scope("proj"):
            for c in range(NPC):
                sl = slice(c * PCHUNK, (c + 1) * PCHUNK)
                xk_c = xchunks.tile([128, 8, PCHUNK], F32R, tag="x", name=f"xk{c}")
                nc.scalar.dma_start(
                    out=xk_c, in_=xkT[:, sl].rearrange("(k p) n -> p k n", p=128))
                for m in range(2):
                    msl = slice(m * 128, (m + 1) * 128)
                    pk = pstp.tile([128, PCHUNK], F32, tag="pst", name=f"pk{c}{m}")
                    for k in range(8):
                        _mm(nc, pk, wk_sb[:, k, msl], xk_c[:, k, :], DT_PROJ,
                            start=(k == 0), stop=(k == 7))
                    nc.vector.tensor_scalar_add(
                        out=kt2[m][:, sl], in0=pk, scalar1=bk_sb[:, m:m + 1])
                xv_c = xchunks.tile([128, 8, PCHUNK], F32R, tag="x", name=f"xv{c}")
                nc.scalar.dma_start(
                    out=xv_c, in_=xvT[:, sl].rearrange("(k p) n -> p k n", p=128))
                for r in range(PCHUNK // 128):
                    rt = c * (PCHUNK // 128) + r
                    rsl = slice(r * 128, (r + 1) * 128)
                    pv = pstp.tile([128, CW], F32, tag="pst", name=f"pv{c}{r}")
                    for k in range(8):
                        _mm(nc, pv, xv_c[:, k, rsl], wv_sb[:, k, :], DT_PROJ,
                            start=(k == 0), stop=(k == 7))
                    for h in range(HPC):
                        nc.vector.tensor_copy(
                            out=vb[h][:, rt, 0:DH],
                            in_=pv[:, h * DH:(h + 1) * DH])
                xq_c = xchunks.tile([128, 8, PCHUNK], F32R, tag="x", name=f"xq{c}")
                nc.scalar.dma_start(
                    out=xq_c, in_=xqT[:, sl].rearrange("(k p) n -> p k n", p=128))
                for m in range(2):
                    msl = slice(m * 128, (m + 1) * 128)
                    pq = pstp.tile([128, PCHUNK], F32, tag="pst", name=f"pq{c}{m}")
                    for k in range(8):
                        _mm(nc, pq, wq_sb[:, k, msl], xq_c[:, k, :], DT_PROJ,
                            start=(k == 0), stop=(k == 7))
                    nc.vector.tensor_scalar_add(
                        out=qt2[m][:, sl], in0=pq, scalar1=bq_sb[:, m:m + 1])

        # ---- phase 2+3: attention (ST/exp stream + dense AV burst) ----
        with nc.named_scope("attn"):
            for qc in range(NQC):
                qsl = slice(qc * QC, (qc + 1) * QC)
                for h in range(HPC):
                    m = h // 2
                    ho = (h % 2) * DH
                    hsl = slice(ho, ho + DH)
                    ets = []
                    for kt in range(KT):
                        ksl = slice(kt * 128, (kt + 1) * 128)
                        pst = pstp.tile([128, QC], F32, tag="pst",
                                        name=f"pst{qc}{h}{kt}")
                        _mm(nc, pst, kt2[m][hsl, ksl], qt2[m][hsl, qsl],
                            DT_SCORE, start=True, stop=True)
                        et_t = work.tile([128, QC], F32R, tag="et", bufs=18,
                                         name=f"et{qc}{h}{kt}")
                        nc.scalar.activation(
                            out=et_t, in_=pst,
                            func=mybir.ActivationFunctionType.Exp, scale=0.125)
                        nc.sync.dma_start(out=et_out[h, ksl, qsl], in_=et_t)
                        ets.append(et_t)
                    pctx = psctx.tile([DH + 1, QC], F32, tag="ctx",
                                      name=f"ctx{qc}{h}")
                    for kt in range(KT):
                        _mm(nc, pctx, vb[h][:, kt, :], ets[kt], DT_AV,
                            start=(kt == 0), stop=(kt == KT - 1))
                    rrow = work.tile([1, QC], F32, tag="rrow",
                                     name=f"rr{qc}{h}")
                    nc.vector.reciprocal(out=rrow, in_=pctx[DH:DH + 1, :])
                    nc.gpsimd.dma_start(out=recip_out[h, qsl], in_=rrow)
                    rs = work.tile([DH, QC], F32, tag="rs", name=f"rs{qc}{h}")
                    nc.gpsimd.partition_broadcast(rs, rrow)
                    nc.vector.tensor_mul(
                        out=ctxT[m][hsl, qsl], in0=pctx[0:DH, :], in1=rs)
                with nc.named_scope("wo"):
                    for q2 in range(QC // 128):
                        qt = qc * (QC // 128) + q2
                        qtsl = slice(qt * 128, (qt + 1) * 128)
                        o_t = work.tile([128, D], F32, tag="ot", name=f"ot{qt}")
                        for n2 in range(2):
                            nsl = slice(n2 * 512, (n2 + 1) * 512)
                            po = pstp.tile([128, 512], F32, tag="pst",
                                           name=f"po{qt}{n2}")
                            for m2 in range(2):
                                _mm(nc, po, ctxT[m2][:, qtsl],
                                    wo_sb[:, m2, nsl], DT_WO,
                                    start=(m2 == 0), stop=(m2 == 1))
                            nc.vector.tensor_copy(out=o_t[:, nsl], in_=po)
                        nc.gpsimd.dma_start(out=outp[qtsl, :], in_=o_t)


_NC_CACHE = None
LAST_RESULTS = None


def _get_nc():
    global _NC_CACHE
    if _NC_CACHE is None:
        _NC_CACHE = build_nc()
    return _NC_CACHE


def kernel(query, key, value, Wq, bq, Wk, bk, Wv, bv, Wo, bo):
    query = np.asarray(query, dtype=np.float32)
    key = np.asarray(key, dtype=np.float32)
    value = np.asarray(value, dtype=np.float32)
    Wq, Wk, Wv, Wo = (np.asarray(a, dtype=np.float32) for a in (Wq, Wk, Wv, Wo))
    bq, bk, bv, bo = (np.asarray(a, dtype=np.float32) for a in (bq, bk, bv, bo))

    nc = _get_nc()

    xT = {}
    for b in range(B):
        xT[b] = (np.ascontiguousarray(query[b].T),
                 np.ascontiguousarray(key[b].T),
                 np.ascontiguousarray(value[b].T))

    in_maps = []
    for c in range(N_CORES):
        b = c // 4
        cb = (c % 4) * CW
        csl = slice(cb, cb + CW)
        in_maps.append({
            "xqT": xT[b][0], "xkT": xT[b][1], "xvT": xT[b][2],
            "wq": np.ascontiguousarray(Wq[:, csl]),
            "wk": np.ascontiguousarray(Wk[:, csl]),
            "wv": np.ascontiguousarray(Wv[:, csl]),
            "bq": np.ascontiguousarray(bq[csl]),
            "bk": np.ascontiguousarray(bk[csl]),
            "wo": np.ascontiguousarray(Wo[csl, :]),
        })

    global LAST_RESULTS
    res = bass_utils.run_bass_kernel_spmd(
        nc, in_maps, core_ids=list(range(N_CORES)))
    LAST_RESULTS = res

    att = np.empty((B, H, S, S), dtype=np.float32)
    out = np.zeros((B, S, D), dtype=np.float32)
    for c in range(N_CORES):
        b = c // 4
        r = res.results[c]
        et = r["et"]          # [HPC, S(k), S(q)]
        recip = r["recip"]    # [HPC, S(q)]
        for hl in range(HPC):
            h = (c % 4) * HPC + hl
            att[b, h] = (et[hl] * recip[hl]).T
        out[b] += r["outp"]
    out += bo + bv @ Wo
    return out, att
